# revision 54
# baseline (speedup 1.0000x reference)
"""Trainium2 Bass kernel for a 2-layer TransformerConv GNN + attention pooling.

Strategy: 64 equal graphs of 128 nodes; edges are within-graph. Shard 8
graphs per NeuronCore (batch sharding). Per graph, the scatter-softmax
attention over edges is computed DENSELY as masked attention with an edge
multiplicity matrix A[dst, src] (ln counts), so everything is PE matmuls.

All heavy matmuls run in fp8e4m3 with the DoubleRow perf mode (0.5 PE
cycles per output row, two 128-deep k-tiles contracted per instruction).
Weights are pre-scaled by 64 on the host so their fp8 grid is centered
(except ws, kept unit so skip + attention share one PSUM scale); the
extra 64x rides through q/k and is folded into the softmax exp scale
(A is host-prescaled by 4096/SCALE to match). Evacuations are spread
round-robin across DVE / ACT / GPSIMD; P^T is produced by XBAR DMA
transposes instead of PE transposes.
"""

import sys
import numpy as np

if "/opt/trn_rl_repo" not in sys.path:
    sys.path.insert(0, "/opt/trn_rl_repo")

import ml_dtypes  # noqa: F401
import concourse.bacc as bacc
import concourse.bass as bass
import concourse.mybir as mybir
import concourse.tile as tile
from concourse.bass_utils import run_bass_kernel_spmd

F32 = mybir.dt.float32
F32R = mybir.dt.float32r
BF16 = mybir.dt.bfloat16
F8 = mybir.dt.float8e4
AF = mybir.ActivationFunctionType
AX = mybir.AxisListType
ALU = mybir.AluOpType
DR = mybir.MatmulPerfMode.DoubleRow

# problem constants (hardcoded per contract)
B, L, D, H, E = 64, 128, 768, 2, 131072
N = B * L                 # 8192 nodes
C = D // H                # 384 per-head channels
N_CORES = 8
G = B // N_CORES          # 8 graphs per core
NPC = G * L               # 1024 nodes per core
KT = D // 128             # 6 k-tiles of 128
KP = KT // 2              # 3 k-tile pairs (DoubleRow)
CT = C // 128             # 3 c-tiles per head
NCH = NPC // 512          # 2 node chunks of 512
SCALE = 1.0 / float(np.sqrt(C))
WS = 64.0                 # fp8 weight/activation scale carried by q/k
EXP_SCALE = SCALE / (WS * WS)
AMASK = -1e35

_CACHE = {}


def _build_program(repeat=1):
    nc = bacc.Bacc("TRN2", target_bir_lowering=False)

    # ---- DRAM I/O ----
    xT_d = nc.dram_tensor("xT", [D, NPC], F8, kind="ExternalInput")
    a_d = nc.dram_tensor("acnt", [G, L, L], BF16, kind="ExternalInput")
    wd = {}
    for l in ("1", "2"):
        for w in ("wq", "wk", "wv", "ws"):
            wd[w + l] = nc.dram_tensor(w + l, [D, D], F8, kind="ExternalInput")
    # all per-channel bias columns in one DMA (HWDGE costs ~625ns per DMA)
    # order: bq64_1 bk64_1 bsv_1 bq64_2 bk64_2 bsv_2 attib64 fc1b2
    bias_d = nc.dram_tensor("bias_pack", [D, 8], F32, kind="ExternalInput")
    atti_w8 = nc.dram_tensor("atti_w8", [2 * D, D], F8, kind="ExternalInput")
    attsw8_d = nc.dram_tensor("attsw8", [D, 1], F8, kind="ExternalInput")
    fc1_w = nc.dram_tensor("fc1_w", [D, D], BF16, kind="ExternalInput")
    fc2_w = nc.dram_tensor("fc2_w", [D, 3], BF16, kind="ExternalInput")
    fc2_b = nc.dram_tensor("fc2_b", [3], F32, kind="ExternalInput")
    eye_b = nc.dram_tensor("eye_b", [128, 128], BF16, kind="ExternalInput")
    tick_d = nc.dram_tensor("tick", [G, 3], F32, kind="ExternalInput")
    out_d = nc.dram_tensor("out", [G, 3], F32, kind="ExternalOutput")

    with tile.TileContext(nc) as tc, (
        tc.tile_pool(name="wpool", bufs=10)) as wp, (
        tc.tile_pool(name="act", bufs=2)) as actp, (
        tc.tile_pool(name="qk", bufs=2)) as qkp, (
        tc.tile_pool(name="small", bufs=1)) as sp, (
        tc.tile_pool(name="attn", bufs=6)) as ap_, (
        tc.tile_pool(name="psum", bufs=6, space=bass.MemorySpace.PSUM)) as pp, (
        tc.tile_pool(name="psumS", bufs=2, space=bass.MemorySpace.PSUM)) as pps:

        # ---- PE warmup: keep the array busy (and ramping) during the
        # initial DMA loads so real matmuls start at full p-state. Also
        # preload the one activation table set the kernel uses
        # (natural_log_exp_and_others: copy/identity/relu/exp/ln).
        warm = sp.tile([128, 128], BF16, tag="warm")
        nc.vector.memset(warm[:], 1.0)
        # pin the activation table to natural_log_exp_and_others (set 6:
        # copy/identity/relu/exp/ln) so no mid-kernel reload is needed;
        # the insert_act_table_loads pass would otherwise flap 0 <-> 5.
        lset = mybir.InstLoadActFuncSet(
            name=nc.get_next_instruction_name(), act_func_set_id=6,
            ins=[], outs=[])
        nc.scalar.add_instruction(lset)
        dummy = sp.tile([1, 1], F32, tag="dummy")
        nc.scalar.activation(dummy[:], warm[0:1, 0:1], AF.Ln)
        nc.scalar.activation(dummy[:], warm[0:1, 0:1], AF.Exp)
        for i in range(30):
            psw = pps.tile([128, 128], F32, tag="sbank")
            nc.tensor.matmul(psw[:], warm[:], warm[:], start=True, stop=True)

        # on-chip constants (no DMA)
        ones_row_f = sp.tile([1, 128], F32, tag="ones_row_f")
        nc.vector.memset(ones_row_f[:], 1.0)
        ones_col_f = sp.tile([128, 1], F32, tag="ones_col_f")
        nc.gpsimd.memset(ones_col_f[:], 1.0)

        # ---- input / weight / constant loads, ordered by first use
        # (single DMA each; HWDGE serializes at ~625ns per DMA).
        xT = actp.tile([128, KT, NPC], F8, tag="xin", bufs=1)
        wq1 = wp.tile([128, KT, D], F8, tag="w", name="wq1")
        for kp in range(KP):  # paired so the first chains start early
            nc.sync.dma_start(
                xT[:, 2 * kp:2 * kp + 2, :],
                xT_d[256 * kp:256 * (kp + 1), :].rearrange(
                    "(t p) n -> p t n", p=128))
            nc.sync.dma_start(
                wq1[:, 2 * kp:2 * kp + 2, :],
                wd["wq1"][256 * kp:256 * (kp + 1), :].rearrange(
                    "(t p) d -> p t d", p=128))
        bp = sp.tile([128, KT, 8], F32, tag="bias_pack")
        nc.sync.dma_start(bp[:], bias_d[:, :].rearrange("(t p) c -> p t c", p=128))
        BIAS = {"bq641": 0, "bk641": 1, "bsv1": 2, "bq642": 3, "bk642": 4,
                "bsv2": 5, "attib": 6, "fc1b2": 7}

        def bcol(name, dt):
            i = BIAS[name]
            return bp[:, dt, i:i + 1]

        def load_w8(dram, name, rows=None):
            t = wp.tile([128, KT, D], F8, tag="w", name=name)
            src = dram if rows is None else dram[rows[0]:rows[1], :]
            nc.sync.dma_start(t[:], src.rearrange("(t p) d -> p t d", p=128))
            return t

        wk1 = load_w8(wd["wk1"], "wk1")
        wv1 = load_w8(wd["wv1"], "wv1")
        ws1 = load_w8(wd["ws1"], "ws1")
        A_sb = sp.tile([128, G, L], BF16, tag="acnt")
        nc.sync.dma_start(A_sb[:], a_d[:, :, :].rearrange("g p s -> p g s"))
        eyeB = sp.tile([128, 128], BF16, tag="eyeB")
        nc.sync.dma_start(eyeB[:], eye_b[:, :])
        wq2 = load_w8(wd["wq2"], "wq2")
        wk2 = load_w8(wd["wk2"], "wk2")
        wv2 = load_w8(wd["wv2"], "wv2")
        ws2 = load_w8(wd["ws2"], "ws2")
        attiT = load_w8(atti_w8, "attiT", rows=(0, D))
        attiB = load_w8(atti_w8, "attiB", rows=(D, 2 * D))
        attsw = sp.tile([128, KT, 1], F8, tag="attsw")
        nc.sync.dma_start(attsw[:],
                          attsw8_d[:, :].rearrange("(t p) o -> p t o", p=128))
        fc1w = wp.tile([128, KT, D], BF16, tag="wb", name="fc1w", bufs=1)
        nc.sync.dma_start(fc1w[:], fc1_w[:, :].rearrange("(t p) d -> p t d", p=128))
        fc2w_c = sp.tile([128, KT, 3], BF16, tag="fc2w")
        nc.sync.dma_start(fc2w_c[:],
                          fc2_w[:, :].rearrange("(t p) o -> p t o", p=128))
        fc2b_c = sp.tile([3, 1], F32, tag="fc2b")
        nc.sync.dma_start(fc2b_c[:], fc2_b[:].rearrange("(o a) -> o a", a=1))
        tick_t = sp.tile([G, 3], F32, tag="tick")
        nc.sync.dma_start(tick_t[:], tick_d[:, :])
        # derived constants
        eye8 = sp.tile([128, 128], F8, tag="eye8")
        nc.vector.tensor_copy(eye8[:], eyeB[:])
        eyeF3 = sp.tile([3, 3], F32, tag="eyeF3")
        nc.gpsimd.tensor_copy(eyeF3[:], eyeB[0:3, 0:3])

        # round-robin PSUM evacuation across DVE / ACT (GPSIMD cannot
        # access PSUM on TRN2 — BIR verifier rejects it).
        def evac_bias(i, out_ap, ps, bcol):
            if i % 2 == 0:
                nc.vector.tensor_scalar(out_ap, ps, bcol, None, ALU.add)
            else:
                nc.scalar.activation(out_ap, ps, AF.Identity, bias=bcol)

        def evac_scale(i, out_ap, ps, s):
            if i % 2 == 0:
                nc.vector.tensor_scalar_mul(out_ap, ps, s)
            else:
                nc.scalar.activation(out_ap, ps, AF.Identity, scale=s)

        def forward():
            # =========== one TransformerConv layer ===========
            def conv_layer(lidx, actT, wq, wk, wv, ws):
                l = str(lidx)
                qT = qkp.tile([128, KT, NPC], F8, tag="qk", name=f"qT{l}")
                kT = qkp.tile([128, KT, NPC], F8, tag="qk", name=f"kT{l}")
                v_sb = qkp.tile([128, G, D], BF16, tag="v", name=f"v{l}")

                # --- q/k (transposed, fp8, carrying x64) + v (natural,
                # bf16, unit scale); evacs round-robin over 3 engines.
                # weight-arrival order: all Q (wq), then K, then V — PE is
                # in-order, so a V chain emitted early would head-of-line
                # block on the wv DMA during layer 1.
                ev = 0
                for w_sb, o_sb, bkey in ((wq, qT, "bq64"), (wk, kT, "bk64")):
                    for dt in range(KT):
                        for ch in range(NCH):
                            ps = pp.tile([128, 512], F32, tag="bank")
                            for kp in range(KP):
                                nc.tensor.matmul(
                                    ps[:],
                                    w_sb[:, 2 * kp:2 * kp + 2,
                                         dt * 128:(dt + 1) * 128],
                                    actT[:, 2 * kp:2 * kp + 2,
                                         ch * 512:(ch + 1) * 512],
                                    start=(kp == 0), stop=(kp == KP - 1),
                                    perf_mode=DR)
                            evac_bias(ev, o_sb[:, dt, ch * 512:(ch + 1) * 512],
                                      ps[:], bcol(bkey + l, dt))
                            ev += 1
                for gv in range(G):
                    for chv in range(2):
                        ps = pp.tile([128, 384], F32, tag="bank")
                        for kp in range(KP):
                            nc.tensor.matmul(
                                ps[:],
                                actT[:, 2 * kp:2 * kp + 2,
                                     gv * 128:(gv + 1) * 128],
                                wv[:, 2 * kp:2 * kp + 2,
                                   chv * 384:(chv + 1) * 384],
                                start=(kp == 0), stop=(kp == KP - 1),
                                perf_mode=DR)
                        evac_scale(ev, v_sb[:, gv, chv * 384:(chv + 1) * 384],
                                   ps[:], 1.0 / WS)
                        ev += 1

                # --- attention + skip, software-pipelined across the 4
                # (chunk, head) blocks: PE runs S(b), T(b-1), skip(b),
                # PV(b-1) back to back so the softmax (ACT/DVE) latency of
                # block b is hidden under skip/PV work of its neighbours.
                hT = actp.tile([128, KT, NPC], F8, tag="act", name=f"hT{l}")
                blocks = [(ch, h) for ch in range(NCH) for h in range(H)]

                def emit_S(b):
                    ch, h = blocks[b]
                    d0 = h * CT
                    psS = pps.tile([128, 512], F32, tag="sbank")
                    for gl in range(4):
                        g = ch * 4 + gl
                        gs = slice(g * 128, (g + 1) * 128)
                        sl = slice(gl * 128, (gl + 1) * 128)
                        nc.tensor.matmul(psS[:, sl], qT[:, d0:d0 + 2, gs],
                                         kT[:, d0:d0 + 2, gs],
                                         start=True, stop=False, perf_mode=DR)
                        nc.tensor.matmul(psS[:, sl], qT[:, d0 + 2, gs],
                                         kT[:, d0 + 2, gs],
                                         start=False, stop=False)
                        nc.tensor.matmul(psS[:, sl], A_sb[:, g, :], eyeB[:],
                                         start=False, stop=True)
                    return psS

                def emit_softmax(psS):
                    # one 512-wide exp on ACT; per-graph row sums and the
                    # normalize run on GPSIMD from SBUF (legal), DVE only
                    # does the tiny max+reciprocal.
                    Pt = ap_.tile([128, 512], BF16, tag="P")
                    Pn = ap_.tile([128, 512], BF16, tag="Pn")
                    nc.scalar.activation(Pt[:], psS[:], AF.Exp,
                                         scale=EXP_SCALE)
                    for gl in range(4):
                        sl = slice(gl * 128, (gl + 1) * 128)
                        Z = ap_.tile([128, 1], F32, tag="Z")
                        nc.vector.tensor_reduce(Z[:], Pt[:, sl], AX.X, ALU.add)
                        nc.vector.tensor_scalar_max(Z[:], Z[:], 1e-30)
                        r = ap_.tile([128, 1], F32, tag="r")
                        nc.vector.reciprocal(r[:], Z[:])
                        nc.gpsimd.tensor_scalar(Pn[:, sl], Pt[:, sl], r[:],
                                                None, ALU.mult)
                    return Pn

                def emit_T(b, Pn):
                    psT = pps.tile([128, 512], BF16, tag="sbank")
                    for gl in range(4):
                        sl = slice(gl * 128, (gl + 1) * 128)
                        nc.tensor.transpose(psT[:, sl], Pn[:, sl], eyeB[:])
                    PT = ap_.tile([128, 512], BF16, tag="PT")
                    nc.scalar.copy(PT[:], psT[:])
                    return PT

                def emit_skip(b):
                    ch, h = blocks[b]
                    d0 = h * CT
                    banks = []
                    for ct in range(CT):
                        dt = d0 + ct
                        ps = pp.tile([128, 512], F32, tag="bank")
                        for kp in range(KP):
                            nc.tensor.matmul(
                                ps[:],
                                ws[:, 2 * kp:2 * kp + 2,
                                   dt * 128:(dt + 1) * 128],
                                actT[:, 2 * kp:2 * kp + 2,
                                     ch * 512:(ch + 1) * 512],
                                start=(kp == 0), stop=False, perf_mode=DR)
                        banks.append(ps)
                    return banks

                def emit_PV_evac(b, PT, banks):
                    ch, h = blocks[b]
                    d0 = h * CT
                    for gl in range(4):
                        g = ch * 4 + gl
                        for ct in range(CT):
                            dti = d0 + ct
                            nc.tensor.matmul(
                                banks[ct][:, gl * 128:(gl + 1) * 128],
                                v_sb[:, g, dti * 128:(dti + 1) * 128],
                                PT[:, gl * 128:(gl + 1) * 128],
                                start=False, stop=(gl == 3))
                    for ct in range(CT):
                        dt = d0 + ct
                        oap = hT[:, dt, ch * 512:(ch + 1) * 512]
                        if ct == 1 or (ct == 2 and b % 2 == 0):
                            nc.vector.tensor_scalar(
                                oap, banks[ct][:],
                                bcol("bsv" + l, dt), 0.0, ALU.add, ALU.max)
                        else:
                            nc.scalar.activation(oap, banks[ct][:], AF.Relu,
                                                 bias=bcol("bsv" + l, dt))

                # iteration order keeps the gating op first in each engine
                # queue: PTcopy(b-1) [DVE/Pool] before softmax(b) [DVE], and
                # exp(b) [ACT] before hT evac(b-1) [ACT].
                prev = None
                for b in range(len(blocks)):
                    psS = emit_S(b)
                    if prev is not None:
                        pb, pPn, pbanks = prev
                        PT = emit_T(pb, pPn)
                        Pn = emit_softmax(psS)
                        banks = emit_skip(b)
                        emit_PV_evac(pb, PT, pbanks)
                    else:
                        Pn = emit_softmax(psS)
                        banks = emit_skip(b)
                    prev = (b, Pn, banks)
                pb, pPn, pbanks = prev
                PT = emit_T(pb, pPn)
                emit_PV_evac(pb, PT, pbanks)
                return hT

            h1T = conv_layer(1, xT, wq1, wk1, wv1, ws1)
            h2T = conv_layer(2, h1T, wq2, wk2, wv2, ws2)

            # =========== attention pooling + head ===========
            h2full = h2T[:]
            pstep = h2full.ap[0][0]

            # h2 natural (bf16) via PE transposes, for pooled = h2^T p.
            # fp8 transpose hardware writes one value per 2-byte cell, so
            # the psum output and the evac read use element-step-2 APs.
            h2n = actp.tile([128, G, D], BF16, tag="h2n", bufs=1)
            ev = 0
            for g in range(G):
                for half, nt in ((0, 4), (1, 2)):
                    psX = pps.tile([128, 2 * nt * 128], F8, tag="sbank")
                    pfull = psX[:]
                    pstepX = pfull.ap[0][0]
                    for j in range(nt):
                        dt = half * 4 + j
                        oap = bass.AP(pfull.tensor, pfull.offset + j * 256,
                                      [[pstepX, 128], [2, 128]])
                        nc.tensor.transpose(
                            oap, h2T[:, dt, g * 128:(g + 1) * 128], eye8[:])
                    rap = bass.AP(pfull.tensor, pfull.offset,
                                  [[pstepX, 128], [256, nt], [2, 128]])
                    evac_scale(ev, h2n[:, g, half * 512:half * 512 + nt * 128],
                               rap, 1.0)
                    ev += 1

            # xc = relu([x_q, h2] @ atti + atti_b)  (x64 scale, fp8).
            # The x_q half contracts against a 0-stride broadcast AP that
            # replays each graph's first-node h2 column 128 times, so no
            # cTb precompute and the evacuation is one wide op per bank.
            xcT = qkp.tile([128, KT, NPC], F8, tag="qk", name="xcT")
            psSc = pp.tile([128, G], F32, tag="bank")
            pooled_sb = sp.tile([128, KT, G], BF16, tag="pooledT")
            ev = 0
            for ch in range(NCH):
                for dt in range(KT):
                    ps = pp.tile([128, 512], F32, tag="bank")
                    for kp in range(KP):
                        nc.tensor.matmul(
                            ps[:],
                            attiB[:, 2 * kp:2 * kp + 2, dt * 128:(dt + 1) * 128],
                            h2T[:, 2 * kp:2 * kp + 2, ch * 512:(ch + 1) * 512],
                            start=(kp == 0), stop=False, perf_mode=DR)
                    for kp in range(KP):
                        xq_bcast = bass.AP(
                            h2full.tensor,
                            h2full.offset + 2 * kp * NPC + ch * 512,
                            [[pstep, 128], [NPC, 2], [L, 4], [0, 128]])
                        nc.tensor.matmul(
                            ps[:],
                            attiT[:, 2 * kp:2 * kp + 2, dt * 128:(dt + 1) * 128],
                            xq_bcast,
                            start=False, stop=(kp == KP - 1), perf_mode=DR)
                    oap = xcT[:, dt, ch * 512:(ch + 1) * 512]
                    if ev % 2 == 0:
                        nc.vector.tensor_scalar(
                            oap, ps[:], bcol("attib", dt), 0.0,
                            ALU.add, ALU.max)
                    else:
                        nc.scalar.activation(oap, ps[:], AF.Relu,
                                             bias=bcol("attib", dt))
                    ev += 1
                # scores (x4096), per-chunk softmax and pooled columns so
                # the tail pipeline starts before the other chunk finishes
                for gl in range(4):
                    g = ch * 4 + gl
                    for kp in range(KP):
                        nc.tensor.matmul(psSc[:, g:g + 1],
                                         xcT[:, 2 * kp:2 * kp + 2,
                                             g * 128:(g + 1) * 128],
                                         attsw[:, 2 * kp:2 * kp + 2, :],
                                         start=(kp == 0), stop=(kp == KP - 1),
                                         perf_mode=DR)
                cs = slice(ch * 4, (ch + 1) * 4)
                Es = ap_.tile([128, 4], F32, tag="Es")
                nc.scalar.activation(Es[:], psSc[:, cs], AF.Exp,
                                     scale=1.0 / (WS * WS))
                psZ = pps.tile([1, 4], F32, tag="sbank")
                nc.tensor.matmul(psZ[:], ones_col_f[:], Es[:],
                                 start=True, stop=True)
                rz = ap_.tile([1, 4], F32, tag="rz")
                nc.vector.reciprocal(rz[:], psZ[:])
                psZb = pps.tile([128, 4], F32, tag="sbank")
                nc.tensor.matmul(psZb[:], ones_row_f[:], rz[:],
                                 start=True, stop=True)
                pcol4 = sp.tile([128, 4], BF16, tag=f"pcols{ch}")
                nc.vector.tensor_mul(pcol4[:], Es[:], psZb[:])
                # pooled columns for this chunk's graphs
                psP = pps.tile([128, KT * 4], F32, tag="sbank")
                for dt in range(KT):
                    for gl in range(4):
                        g = ch * 4 + gl
                        nc.tensor.matmul(psP[:, dt * 4 + gl:dt * 4 + gl + 1],
                                         h2n[:, g, dt * 128:(dt + 1) * 128],
                                         pcol4[:, gl:gl + 1],
                                         start=True, stop=True)
                pf = pooled_sb[:]
                oap = bass.AP(pf.tensor, pf.offset + ch * 4,
                              [[pf.ap[0][0], 128], [G, KT], [1, 4]])
                nc.scalar.copy(oap, psP[:])

            pooledT = pooled_sb

            # fc1 + tanh (transposed); tanh computed as 1 - 2/(e^{2x}+1)
            # so the kernel only ever needs the exp/ln activation table set
            # (saves two LoadActFuncSet reloads in the serial tail).
            z1 = sp.tile([128, KT, G], BF16, tag="z1")
            for dt in range(KT):
                ps = pps.tile([128, G], F32, tag="sbank")
                for kt in range(KT):
                    nc.tensor.matmul(ps[:],
                                     fc1w[:, kt, dt * 128:(dt + 1) * 128],
                                     pooledT[:, kt, :],
                                     start=(kt == 0), stop=(kt == KT - 1))
                e1 = ap_.tile([128, G], F32, tag="e1")
                nc.scalar.activation(e1[:], ps[:], AF.Exp, scale=2.0,
                                     bias=bcol("fc1b2", dt))
                ep = ap_.tile([128, G], F32, tag="ep")
                nc.vector.tensor_scalar_add(ep[:], e1[:], 1.0)
                rc = ap_.tile([128, G], F32, tag="rc")
                nc.vector.reciprocal(rc[:], ep[:])
                nc.vector.tensor_scalar(z1[:, dt, :], rc[:], -2.0, 1.0,
                                        ALU.mult, ALU.add)

            # fc2 -> [3, G] -> transpose -> log_softmax -> out
            psO = pps.tile([3, G], F32, tag="sbank")
            for kt in range(KT):
                nc.tensor.matmul(psO[:], fc2w_c[:, kt, :], z1[:, kt, :],
                                 start=(kt == 0), stop=(kt == KT - 1))
            oT = sp.tile([3, G], F32, tag="oT")
            nc.scalar.activation(oT[:], psO[:], AF.Identity, bias=fc2b_c[:])
            psOt = pps.tile([G, 3], F32, tag="sbank")
            nc.tensor.transpose(psOt[:], oT[:], eyeF3[:])
            # logits are O(1): log_softmax without max-subtraction
            eo = ap_.tile([G, 3], F32, tag="eo")
            zo = ap_.tile([G, 1], F32, tag="zo")
            nc.scalar.activation(eo[:], psOt[:], AF.Exp, accum_out=zo[:])
            lz = ap_.tile([G, 1], F32, tag="lz")
            nc.scalar.activation(lz[:], zo[:], AF.Ln)
            ofin = ap_.tile([G, 3], F32, tag="ofin")
            nc.vector.tensor_scalar(ofin[:], psOt[:], lz[:], None, ALU.subtract)
            nc.sync.dma_start(out_d[:, :], ofin[:])

        for _ in range(repeat):
            forward()

    nc.compile()
    return nc


def _get_program(repeat=1):
    key = ("nc", repeat)
    if key not in _CACHE:
        _CACHE[key] = _build_program(repeat)
    return _CACHE[key]


def make_in_maps(inputs):
    F8N = ml_dtypes.float8_e4m3
    BFN = ml_dtypes.bfloat16
    x = np.asarray(inputs["x"], np.float32)
    ei = np.asarray(inputs["edge_index"])
    src, dst = ei[0].astype(np.int64), ei[1].astype(np.int64)
    # A[graph, src_local, dst_local]: ln(edge count) pre-scaled to cancel the
    # x64 q/k scales inside exp; -1e35 masks non-edges.
    flat = dst * L + (src % L)
    acnt = np.bincount(flat, minlength=N * L).reshape(B, L, L).astype(np.float32)
    with np.errstate(divide="ignore"):
        aval = np.where(acnt > 0, np.log(acnt) * (WS * WS / SCALE),
                        np.float32(AMASK))
    aval = np.ascontiguousarray(aval.transpose(0, 2, 1)).astype(BFN)

    shared = {}
    for l in ("1", "2"):
        for w in ("wq", "wk", "wv"):
            shared[w + l] = (np.asarray(inputs[w + l], np.float32) * WS
                             ).astype(F8N)
        shared["ws" + l] = np.asarray(inputs["ws" + l], np.float32).astype(F8N)
    # bias_pack columns: bq64_1 bk64_1 bsv_1 bq64_2 bk64_2 bsv_2 attib64 fc1b2
    cols = []
    for l in ("1", "2"):
        cols.append(np.asarray(inputs["bq" + l], np.float32) * WS)
        cols.append(np.asarray(inputs["bk" + l], np.float32) * WS)
        cols.append(np.asarray(inputs["bs" + l], np.float32)
                    + np.asarray(inputs["bv" + l], np.float32))
    cols = [cols[0], cols[1], cols[2], cols[3], cols[4], cols[5],
            np.asarray(inputs["atti_b"], np.float32) * WS,
            np.asarray(inputs["fc1_b"], np.float32) * 2.0]
    shared["bias_pack"] = np.ascontiguousarray(np.stack(cols, axis=1))
    shared["atti_w8"] = (np.asarray(inputs["atti_w"], np.float32) * WS
                         ).astype(F8N)
    shared["attsw8"] = (np.asarray(inputs["atts_w"], np.float32) * WS
                        ).astype(F8N)
    shared["fc1_w"] = np.asarray(inputs["fc1_w"], np.float32).astype(BFN)
    shared["fc2_w"] = np.asarray(inputs["fc2_w"], np.float32).astype(BFN)
    shared["fc2_b"] = np.asarray(inputs["fc2_b"], np.float32)
    shared["eye_b"] = np.eye(128, dtype=BFN)

    in_maps = []
    for c in range(N_CORES):
        m = dict(shared)
        m["tick"] = np.zeros((G, 3), np.float32)
        m["xT"] = np.ascontiguousarray(
            x[c * NPC:(c + 1) * NPC].T).astype(F8N)
        m["acnt"] = np.ascontiguousarray(aval[c * G:(c + 1) * G])
        in_maps.append(m)
    return in_maps


def kernel(**inputs):
    nc = _get_program()
    in_maps = make_in_maps(inputs)
    res = run_bass_kernel_spmd(nc, in_maps, core_ids=list(range(N_CORES)))
    out = np.concatenate([res.results[c]["out"] for c in range(N_CORES)], axis=0)
    return out.astype(np.float32)


# revision 61
# speedup vs baseline: 3.8618x; 3.8618x over previous
"""Trainium2 Bass kernel for a 2-layer TransformerConv GNN + attention pooling.

Strategy: 64 equal graphs of 128 nodes; edges are within-graph. Shard 8
graphs per NeuronCore (batch sharding). Per graph, the scatter-softmax
attention over edges is computed DENSELY as masked attention with an edge
multiplicity matrix A[dst, src] (ln counts), so everything is PE matmuls.

All heavy matmuls run in fp8e4m3 with the DoubleRow perf mode (0.5 PE
cycles per output row, two 128-deep k-tiles contracted per instruction).
Weights are pre-scaled by 64 on the host so their fp8 grid is centered
(except ws, kept unit so skip + attention share one PSUM scale); the
extra 64x rides through q/k and is folded into the softmax exp scale
(A is host-prescaled by 4096/SCALE to match). The attention blocks are
software-pipelined (S(b) | P^T(b-1) | skip(b) | PV(b-1)) so softmax
latency hides under PE work; PSUM evacuations alternate DVE/ACT
(GPSIMD cannot access PSUM) with SBUF-side softmax ops on GPSIMD.
The x_q pooling term is contracted via a 0-stride broadcast AP, and
tanh is computed from exp so one activation-table set serves the whole
kernel (pinned up front; no mid-kernel table reloads).
"""

import sys
import numpy as np

if "/opt/trn_rl_repo" not in sys.path:
    sys.path.insert(0, "/opt/trn_rl_repo")

import ml_dtypes  # noqa: F401
import concourse.bacc as bacc
import concourse.bass as bass
import concourse.mybir as mybir
import concourse.tile as tile
from concourse.bass_utils import run_bass_kernel_spmd

F32 = mybir.dt.float32
F32R = mybir.dt.float32r
BF16 = mybir.dt.bfloat16
F8 = mybir.dt.float8e4
AF = mybir.ActivationFunctionType
AX = mybir.AxisListType
ALU = mybir.AluOpType
DR = mybir.MatmulPerfMode.DoubleRow

# problem constants (hardcoded per contract)
B, L, D, H, E = 64, 128, 768, 2, 131072
N = B * L                 # 8192 nodes
C = D // H                # 384 per-head channels
N_CORES = 8
G = B // N_CORES          # 8 graphs per core
NPC = G * L               # 1024 nodes per core
KT = D // 128             # 6 k-tiles of 128
KP = KT // 2              # 3 k-tile pairs (DoubleRow)
CT = C // 128             # 3 c-tiles per head
NCH = NPC // 512          # 2 node chunks of 512
SCALE = 1.0 / float(np.sqrt(C))
WS = 64.0                 # fp8 weight/activation scale carried by q/k
EXP_SCALE = SCALE / (WS * WS)
AMASK = -1e35

_CACHE = {}


def _build_program(repeat=1):
    nc = bacc.Bacc("TRN2", target_bir_lowering=False)

    # ---- DRAM I/O ----
    xT_d = nc.dram_tensor("xT", [D, NPC], F8, kind="ExternalInput")
    a_d = nc.dram_tensor("acnt", [G, L, L], BF16, kind="ExternalInput")
    wd = {}
    for l in ("1", "2"):
        for w in ("wq", "wk", "wv", "ws"):
            wd[w + l] = nc.dram_tensor(w + l, [D, D], F8, kind="ExternalInput")
    # all per-channel bias columns in one DMA (HWDGE costs ~625ns per DMA)
    # order: bq64_1 bk64_1 bsv_1 bq64_2 bk64_2 bsv_2 attib64 fc1b2
    bias_d = nc.dram_tensor("bias_pack", [D, 8], F32, kind="ExternalInput")
    atti_w8 = nc.dram_tensor("atti_w8", [2 * D, D], F8, kind="ExternalInput")
    attsw8_d = nc.dram_tensor("attsw8", [D, 1], F8, kind="ExternalInput")
    fc1_w = nc.dram_tensor("fc1_w", [D, D], BF16, kind="ExternalInput")
    fc2_w = nc.dram_tensor("fc2_w", [D, 3], BF16, kind="ExternalInput")
    fc2_b = nc.dram_tensor("fc2_b", [3], F32, kind="ExternalInput")
    eye_b = nc.dram_tensor("eye_b", [128, 128], BF16, kind="ExternalInput")
    tick_d = nc.dram_tensor("tick", [G, 3], F32, kind="ExternalInput")
    out_d = nc.dram_tensor("out", [G, 3], F32, kind="ExternalOutput")

    with tile.TileContext(nc) as tc, (
        tc.tile_pool(name="wpool", bufs=10)) as wp, (
        tc.tile_pool(name="act", bufs=2)) as actp, (
        tc.tile_pool(name="qk", bufs=2)) as qkp, (
        tc.tile_pool(name="small", bufs=1)) as sp, (
        tc.tile_pool(name="attn", bufs=6)) as ap_, (
        tc.tile_pool(name="psum", bufs=6, space=bass.MemorySpace.PSUM)) as pp, (
        tc.tile_pool(name="psumS", bufs=2, space=bass.MemorySpace.PSUM)) as pps:

        # ---- PE warmup: keep the array busy (and ramping) during the
        # initial DMA loads so real matmuls start at full p-state. Also
        # preload the one activation table set the kernel uses
        # (natural_log_exp_and_others: copy/identity/relu/exp/ln).
        warm = sp.tile([128, 128], BF16, tag="warm")
        nc.vector.memset(warm[:], 1.0)
        # pin the activation table to natural_log_exp_and_others (set 6:
        # copy/identity/relu/exp/ln) so no mid-kernel reload is needed;
        # the insert_act_table_loads pass would otherwise flap 0 <-> 5.
        lset = mybir.InstLoadActFuncSet(
            name=nc.get_next_instruction_name(), act_func_set_id=6,
            ins=[], outs=[])
        nc.scalar.add_instruction(lset)
        dummy = sp.tile([1, 1], F32, tag="dummy")
        nc.scalar.activation(dummy[:], warm[0:1, 0:1], AF.Ln)
        nc.scalar.activation(dummy[:], warm[0:1, 0:1], AF.Exp)
        for i in range(30):
            psw = pps.tile([128, 128], F32, tag="sbank")
            nc.tensor.matmul(psw[:], warm[:], warm[:], start=True, stop=True)

        # on-chip constants (no DMA)
        ones_row_f = sp.tile([1, 128], F32, tag="ones_row_f")
        nc.vector.memset(ones_row_f[:], 1.0)
        ones_col_f = sp.tile([128, 1], F32, tag="ones_col_f")
        nc.gpsimd.memset(ones_col_f[:], 1.0)

        # ---- input / weight / constant loads, ordered by first use
        # (single DMA each; HWDGE serializes at ~625ns per DMA).
        xT = actp.tile([128, KT, NPC], F8, tag="xin", bufs=1)
        wq1 = wp.tile([128, KT, D], F8, tag="w", name="wq1")
        for kp in range(KP):  # paired so the first chains start early
            nc.sync.dma_start(
                xT[:, 2 * kp:2 * kp + 2, :],
                xT_d[256 * kp:256 * (kp + 1), :].rearrange(
                    "(t p) n -> p t n", p=128))
            nc.sync.dma_start(
                wq1[:, 2 * kp:2 * kp + 2, :],
                wd["wq1"][256 * kp:256 * (kp + 1), :].rearrange(
                    "(t p) d -> p t d", p=128))
        bp = sp.tile([128, KT, 8], F32, tag="bias_pack")
        nc.sync.dma_start(bp[:], bias_d[:, :].rearrange("(t p) c -> p t c", p=128))
        BIAS = {"bq641": 0, "bk641": 1, "bsv1": 2, "bq642": 3, "bk642": 4,
                "bsv2": 5, "attib": 6, "fc1b2": 7}

        def bcol(name, dt):
            i = BIAS[name]
            return bp[:, dt, i:i + 1]

        def load_w8(dram, name, rows=None):
            t = wp.tile([128, KT, D], F8, tag="w", name=name)
            src = dram if rows is None else dram[rows[0]:rows[1], :]
            nc.sync.dma_start(t[:], src.rearrange("(t p) d -> p t d", p=128))
            return t

        wk1 = load_w8(wd["wk1"], "wk1")
        wv1 = load_w8(wd["wv1"], "wv1")
        ws1 = load_w8(wd["ws1"], "ws1")
        A_sb = sp.tile([128, G, L], BF16, tag="acnt")
        nc.sync.dma_start(A_sb[:], a_d[:, :, :].rearrange("g p s -> p g s"))
        eyeB = sp.tile([128, 128], BF16, tag="eyeB")
        nc.sync.dma_start(eyeB[:], eye_b[:, :])
        wq2 = load_w8(wd["wq2"], "wq2")
        wk2 = load_w8(wd["wk2"], "wk2")
        wv2 = load_w8(wd["wv2"], "wv2")
        ws2 = load_w8(wd["ws2"], "ws2")
        attiT = load_w8(atti_w8, "attiT", rows=(0, D))
        attiB = load_w8(atti_w8, "attiB", rows=(D, 2 * D))
        attsw = sp.tile([128, KT, 1], F8, tag="attsw")
        nc.sync.dma_start(attsw[:],
                          attsw8_d[:, :].rearrange("(t p) o -> p t o", p=128))
        fc1w = wp.tile([128, KT, D], BF16, tag="wb", name="fc1w", bufs=1)
        nc.sync.dma_start(fc1w[:], fc1_w[:, :].rearrange("(t p) d -> p t d", p=128))
        fc2w_c = sp.tile([128, KT, 3], BF16, tag="fc2w")
        nc.sync.dma_start(fc2w_c[:],
                          fc2_w[:, :].rearrange("(t p) o -> p t o", p=128))
        fc2b_c = sp.tile([3, 1], F32, tag="fc2b")
        nc.sync.dma_start(fc2b_c[:], fc2_b[:].rearrange("(o a) -> o a", a=1))
        tick_t = sp.tile([G, 3], F32, tag="tick")
        nc.sync.dma_start(tick_t[:], tick_d[:, :])
        # derived constants
        eye8 = sp.tile([128, 128], F8, tag="eye8")
        nc.vector.tensor_copy(eye8[:], eyeB[:])
        eyeF3 = sp.tile([3, 3], F32, tag="eyeF3")
        nc.gpsimd.tensor_copy(eyeF3[:], eyeB[0:3, 0:3])

        # round-robin PSUM evacuation across DVE / ACT (GPSIMD cannot
        # access PSUM on TRN2 — BIR verifier rejects it).
        def evac_bias(i, out_ap, ps, bcol):
            if i % 2 == 0:
                nc.vector.tensor_scalar(out_ap, ps, bcol, None, ALU.add)
            else:
                nc.scalar.activation(out_ap, ps, AF.Identity, bias=bcol)

        def evac_scale(i, out_ap, ps, s):
            if i % 2 == 0:
                nc.vector.tensor_scalar_mul(out_ap, ps, s)
            else:
                nc.scalar.activation(out_ap, ps, AF.Identity, scale=s)

        def forward():
            # =========== one TransformerConv layer ===========
            def conv_layer(lidx, actT, wq, wk, wv, ws):
                l = str(lidx)
                qT = qkp.tile([128, KT, NPC], F8, tag="qk", name=f"qT{l}")
                kT = qkp.tile([128, KT, NPC], F8, tag="qk", name=f"kT{l}")
                v_sb = qkp.tile([128, G, D], BF16, tag="v", name=f"v{l}")

                # --- q/k (transposed, fp8, carrying x64) + v (natural,
                # bf16, unit scale); evacs round-robin over 3 engines.
                # weight-arrival order: all Q (wq), then K, then V — PE is
                # in-order, so a V chain emitted early would head-of-line
                # block on the wv DMA during layer 1.
                ev = 0
                for w_sb, o_sb, bkey in ((wq, qT, "bq64"), (wk, kT, "bk64")):
                    for dt in range(KT):
                        for ch in range(NCH):
                            ps = pp.tile([128, 512], F32, tag="bank")
                            for kp in range(KP):
                                nc.tensor.matmul(
                                    ps[:],
                                    w_sb[:, 2 * kp:2 * kp + 2,
                                         dt * 128:(dt + 1) * 128],
                                    actT[:, 2 * kp:2 * kp + 2,
                                         ch * 512:(ch + 1) * 512],
                                    start=(kp == 0), stop=(kp == KP - 1),
                                    perf_mode=DR)
                            evac_bias(ev, o_sb[:, dt, ch * 512:(ch + 1) * 512],
                                      ps[:], bcol(bkey + l, dt))
                            ev += 1
                for gv in range(G):
                    for chv in range(2):
                        ps = pp.tile([128, 384], F32, tag="bank")
                        for kp in range(KP):
                            nc.tensor.matmul(
                                ps[:],
                                actT[:, 2 * kp:2 * kp + 2,
                                     gv * 128:(gv + 1) * 128],
                                wv[:, 2 * kp:2 * kp + 2,
                                   chv * 384:(chv + 1) * 384],
                                start=(kp == 0), stop=(kp == KP - 1),
                                perf_mode=DR)
                        evac_scale(ev, v_sb[:, gv, chv * 384:(chv + 1) * 384],
                                   ps[:], 1.0 / WS)
                        ev += 1

                # --- attention + skip, software-pipelined across the 4
                # (chunk, head) blocks: PE runs S(b), T(b-1), skip(b),
                # PV(b-1) back to back so the softmax (ACT/DVE) latency of
                # block b is hidden under skip/PV work of its neighbours.
                hT = actp.tile([128, KT, NPC], F8, tag="act", name=f"hT{l}")
                blocks = [(ch, h) for ch in range(NCH) for h in range(H)]

                def emit_S(b):
                    ch, h = blocks[b]
                    d0 = h * CT
                    psS = pps.tile([128, 512], F32, tag="sbank")
                    for gl in range(4):
                        g = ch * 4 + gl
                        gs = slice(g * 128, (g + 1) * 128)
                        sl = slice(gl * 128, (gl + 1) * 128)
                        nc.tensor.matmul(psS[:, sl], qT[:, d0:d0 + 2, gs],
                                         kT[:, d0:d0 + 2, gs],
                                         start=True, stop=False, perf_mode=DR)
                        nc.tensor.matmul(psS[:, sl], qT[:, d0 + 2, gs],
                                         kT[:, d0 + 2, gs],
                                         start=False, stop=False)
                        nc.tensor.matmul(psS[:, sl], A_sb[:, g, :], eyeB[:],
                                         start=False, stop=True)
                    return psS

                def emit_softmax(psS):
                    # one 512-wide exp on ACT; per-graph row sums and the
                    # normalize run on GPSIMD from SBUF (legal), DVE only
                    # does the tiny max+reciprocal.
                    Pt = ap_.tile([128, 512], BF16, tag="P")
                    Pn = ap_.tile([128, 512], BF16, tag="Pn")
                    nc.scalar.activation(Pt[:, 0:256], psS[:, 0:256], AF.Exp,
                                         scale=EXP_SCALE)
                    nc.scalar.activation(Pt[:, 256:512], psS[:, 256:512],
                                         AF.Exp, scale=EXP_SCALE)
                    for gl in range(4):
                        sl = slice(gl * 128, (gl + 1) * 128)
                        Z = ap_.tile([128, 1], F32, tag="Z")
                        nc.vector.tensor_reduce(Z[:], Pt[:, sl], AX.X, ALU.add)
                        nc.vector.tensor_scalar_max(Z[:], Z[:], 1e-30)
                        r = ap_.tile([128, 1], F32, tag="r")
                        nc.vector.reciprocal(r[:], Z[:])
                        nc.gpsimd.tensor_scalar(Pn[:, sl], Pt[:, sl], r[:],
                                                None, ALU.mult)
                    return Pn

                def emit_T(b, Pn):
                    psT = pps.tile([128, 512], BF16, tag="sbank")
                    for gl in range(4):
                        sl = slice(gl * 128, (gl + 1) * 128)
                        nc.tensor.transpose(psT[:, sl], Pn[:, sl], eyeB[:])
                    PT = ap_.tile([128, 512], BF16, tag="PT")
                    nc.scalar.copy(PT[:], psT[:])
                    return PT

                def emit_skip(b):
                    ch, h = blocks[b]
                    d0 = h * CT
                    banks = []
                    for ct in range(CT):
                        dt = d0 + ct
                        ps = pp.tile([128, 512], F32, tag="bank")
                        for kp in range(KP):
                            nc.tensor.matmul(
                                ps[:],
                                ws[:, 2 * kp:2 * kp + 2,
                                   dt * 128:(dt + 1) * 128],
                                actT[:, 2 * kp:2 * kp + 2,
                                     ch * 512:(ch + 1) * 512],
                                start=(kp == 0), stop=False, perf_mode=DR)
                        banks.append(ps)
                    return banks

                def emit_PV_evac(b, PT, banks):
                    ch, h = blocks[b]
                    d0 = h * CT
                    for gl in range(4):
                        g = ch * 4 + gl
                        for ct in range(CT):
                            dti = d0 + ct
                            nc.tensor.matmul(
                                banks[ct][:, gl * 128:(gl + 1) * 128],
                                v_sb[:, g, dti * 128:(dti + 1) * 128],
                                PT[:, gl * 128:(gl + 1) * 128],
                                start=False, stop=(gl == 3))
                    for ct in range(CT):
                        dt = d0 + ct
                        oap = hT[:, dt, ch * 512:(ch + 1) * 512]
                        if ct == 1 or (ct == 2 and b % 2 == 0):
                            nc.vector.tensor_scalar(
                                oap, banks[ct][:],
                                bcol("bsv" + l, dt), 0.0, ALU.add, ALU.max)
                        else:
                            nc.scalar.activation(oap, banks[ct][:], AF.Relu,
                                                 bias=bcol("bsv" + l, dt))

                # iteration order keeps the gating op first in each engine
                # queue: PTcopy(b-1) [DVE/Pool] before softmax(b) [DVE], and
                # exp(b) [ACT] before hT evac(b-1) [ACT].
                prev = None
                for b in range(len(blocks)):
                    psS = emit_S(b)
                    if prev is not None:
                        pb, pPn, pbanks = prev
                        PT = emit_T(pb, pPn)
                        Pn = emit_softmax(psS)
                        banks = emit_skip(b)
                        emit_PV_evac(pb, PT, pbanks)
                    else:
                        Pn = emit_softmax(psS)
                        banks = emit_skip(b)
                    prev = (b, Pn, banks)
                pb, pPn, pbanks = prev
                PT = emit_T(pb, pPn)
                emit_PV_evac(pb, PT, pbanks)
                return hT

            h1T = conv_layer(1, xT, wq1, wk1, wv1, ws1)
            h2T = conv_layer(2, h1T, wq2, wk2, wv2, ws2)

            # =========== attention pooling + head ===========
            h2full = h2T[:]
            pstep = h2full.ap[0][0]

            # h2 natural (bf16) via PE transposes, for pooled = h2^T p.
            # fp8 transpose hardware writes one value per 2-byte cell, so
            # the psum output and the evac read use element-step-2 APs.
            h2n = actp.tile([128, G, D], BF16, tag="h2n", bufs=1)
            ev = 0
            for g in range(G):
                for half, nt in ((0, 4), (1, 2)):
                    psX = pps.tile([128, 2 * nt * 128], F8, tag="sbank")
                    pfull = psX[:]
                    pstepX = pfull.ap[0][0]
                    for j in range(nt):
                        dt = half * 4 + j
                        oap = bass.AP(pfull.tensor, pfull.offset + j * 256,
                                      [[pstepX, 128], [2, 128]])
                        nc.tensor.transpose(
                            oap, h2T[:, dt, g * 128:(g + 1) * 128], eye8[:])
                    rap = bass.AP(pfull.tensor, pfull.offset,
                                  [[pstepX, 128], [256, nt], [2, 128]])
                    evac_scale(ev, h2n[:, g, half * 512:half * 512 + nt * 128],
                               rap, 1.0)
                    ev += 1

            # xc = relu([x_q, h2] @ atti + atti_b)  (x64 scale, fp8).
            # The x_q half contracts against a 0-stride broadcast AP that
            # replays each graph's first-node h2 column 128 times, so no
            # cTb precompute and the evacuation is one wide op per bank.
            xcT = qkp.tile([128, KT, NPC], F8, tag="qk", name="xcT")
            psSc = pp.tile([128, G], F32, tag="bank")
            pooled_sb = sp.tile([128, KT, G], BF16, tag="pooledT")
            z1 = sp.tile([128, KT, G], BF16, tag="z1")
            ev = 0
            for ch in range(NCH):
                for dt in range(KT):
                    ps = pp.tile([128, 512], F32, tag="bank")
                    for kp in range(KP):
                        nc.tensor.matmul(
                            ps[:],
                            attiB[:, 2 * kp:2 * kp + 2, dt * 128:(dt + 1) * 128],
                            h2T[:, 2 * kp:2 * kp + 2, ch * 512:(ch + 1) * 512],
                            start=(kp == 0), stop=False, perf_mode=DR)
                    for kp in range(KP):
                        xq_bcast = bass.AP(
                            h2full.tensor,
                            h2full.offset + 2 * kp * NPC + ch * 512,
                            [[pstep, 128], [NPC, 2], [L, 4], [0, 128]])
                        nc.tensor.matmul(
                            ps[:],
                            attiT[:, 2 * kp:2 * kp + 2, dt * 128:(dt + 1) * 128],
                            xq_bcast,
                            start=False, stop=(kp == KP - 1), perf_mode=DR)
                    oap = xcT[:, dt, ch * 512:(ch + 1) * 512]
                    if ev % 2 == 0:
                        nc.vector.tensor_scalar(
                            oap, ps[:], bcol("attib", dt), 0.0,
                            ALU.add, ALU.max)
                    else:
                        nc.scalar.activation(oap, ps[:], AF.Relu,
                                             bias=bcol("attib", dt))
                    ev += 1
                # scores (x4096), per-chunk softmax and pooled columns so
                # the tail pipeline starts before the other chunk finishes
                for gl in range(4):
                    g = ch * 4 + gl
                    for kp in range(KP):
                        nc.tensor.matmul(psSc[:, g:g + 1],
                                         xcT[:, 2 * kp:2 * kp + 2,
                                             g * 128:(g + 1) * 128],
                                         attsw[:, 2 * kp:2 * kp + 2, :],
                                         start=(kp == 0), stop=(kp == KP - 1),
                                         perf_mode=DR)
                cs = slice(ch * 4, (ch + 1) * 4)
                Es = ap_.tile([128, 4], F32, tag="Es")
                nc.scalar.activation(Es[:], psSc[:, cs], AF.Exp,
                                     scale=1.0 / (WS * WS))
                psZ = pps.tile([1, 4], F32, tag="sbank")
                nc.tensor.matmul(psZ[:], ones_col_f[:], Es[:],
                                 start=True, stop=True)
                rz = ap_.tile([1, 4], F32, tag="rz")
                nc.vector.reciprocal(rz[:], psZ[:])
                psZb = pps.tile([128, 4], F32, tag="sbank")
                nc.tensor.matmul(psZb[:], ones_row_f[:], rz[:],
                                 start=True, stop=True)
                pcol4 = sp.tile([128, 4], BF16, tag=f"pcols{ch}")
                nc.vector.tensor_mul(pcol4[:], Es[:], psZb[:])
                # pooled columns for this chunk's graphs
                psP = pps.tile([128, KT * 4], F32, tag="sbank")
                for dt in range(KT):
                    for gl in range(4):
                        g = ch * 4 + gl
                        nc.tensor.matmul(psP[:, dt * 4 + gl:dt * 4 + gl + 1],
                                         h2n[:, g, dt * 128:(dt + 1) * 128],
                                         pcol4[:, gl:gl + 1],
                                         start=True, stop=True)
                pf = pooled_sb[:]
                oap = bass.AP(pf.tensor, pf.offset + ch * 4,
                              [[pf.ap[0][0], 128], [G, KT], [1, 4]])
                nc.scalar.copy(oap, psP[:])

            # fc1 + tanh (as 1 - 2/(e^{2x}+1): keeps the single exp/ln
            # activation table set)
            for dt in range(KT):
                ps = pps.tile([128, G], F32, tag="sbank")
                for kt in range(KT):
                    nc.tensor.matmul(ps[:],
                                     fc1w[:, kt, dt * 128:(dt + 1) * 128],
                                     pooled_sb[:, kt, :],
                                     start=(kt == 0), stop=(kt == KT - 1))
                e1 = ap_.tile([128, G], F32, tag="e1")
                nc.scalar.activation(e1[:], ps[:], AF.Exp, scale=2.0,
                                     bias=bcol("fc1b2", dt))
                ep = ap_.tile([128, G], F32, tag="ep")
                nc.vector.tensor_scalar_add(ep[:], e1[:], 1.0)
                rc = ap_.tile([128, G], F32, tag="rc")
                nc.vector.reciprocal(rc[:], ep[:])
                nc.vector.tensor_scalar(z1[:, dt, :], rc[:], -2.0, 1.0,
                                        ALU.mult, ALU.add)

            # fc2 -> [3, G]
            psO = pps.tile([3, G], F32, tag="sbank")
            for kt in range(KT):
                nc.tensor.matmul(psO[:], fc2w_c[:, kt, :], z1[:, kt, :],
                                 start=(kt == 0), stop=(kt == KT - 1))
            oT = sp.tile([3, G], F32, tag="oT")
            nc.scalar.activation(oT[:], psO[:], AF.Identity, bias=fc2b_c[:])
            psOt = pps.tile([G, 3], F32, tag="sbank")
            nc.tensor.transpose(psOt[:], oT[:], eyeF3[:])
            # logits are O(1): log_softmax without max-subtraction
            eo = ap_.tile([G, 3], F32, tag="eo")
            zo = ap_.tile([G, 1], F32, tag="zo")
            nc.scalar.activation(eo[:], psOt[:], AF.Exp, accum_out=zo[:])
            lz = ap_.tile([G, 1], F32, tag="lz")
            nc.scalar.activation(lz[:], zo[:], AF.Ln)
            ofin = ap_.tile([G, 3], F32, tag="ofin")
            nc.vector.tensor_scalar(ofin[:], psOt[:], lz[:], None, ALU.subtract)
            nc.sync.dma_start(out_d[:, :], ofin[:])

        for _ in range(repeat):
            forward()

    nc.compile()
    return nc


def _get_program(repeat=1):
    key = ("nc", repeat)
    if key not in _CACHE:
        _CACHE[key] = _build_program(repeat)
    return _CACHE[key]


def make_in_maps(inputs):
    F8N = ml_dtypes.float8_e4m3
    BFN = ml_dtypes.bfloat16
    x = np.asarray(inputs["x"], np.float32)
    ei = np.asarray(inputs["edge_index"])
    src, dst = ei[0].astype(np.int64), ei[1].astype(np.int64)
    # A[graph, src_local, dst_local]: ln(edge count) pre-scaled to cancel the
    # x64 q/k scales inside exp; -1e35 masks non-edges.
    flat = dst * L + (src % L)
    acnt = np.bincount(flat, minlength=N * L).reshape(B, L, L).astype(np.float32)
    with np.errstate(divide="ignore"):
        aval = np.where(acnt > 0, np.log(acnt) * (WS * WS / SCALE),
                        np.float32(AMASK))
    aval = np.ascontiguousarray(aval.transpose(0, 2, 1)).astype(BFN)

    shared = {}
    for l in ("1", "2"):
        for w in ("wq", "wk", "wv"):
            shared[w + l] = (np.asarray(inputs[w + l], np.float32) * WS
                             ).astype(F8N)
        shared["ws" + l] = np.asarray(inputs["ws" + l], np.float32).astype(F8N)
    # bias_pack columns: bq64_1 bk64_1 bsv_1 bq64_2 bk64_2 bsv_2 attib64 fc1b2
    cols = []
    for l in ("1", "2"):
        cols.append(np.asarray(inputs["bq" + l], np.float32) * WS)
        cols.append(np.asarray(inputs["bk" + l], np.float32) * WS)
        cols.append(np.asarray(inputs["bs" + l], np.float32)
                    + np.asarray(inputs["bv" + l], np.float32))
    cols = [cols[0], cols[1], cols[2], cols[3], cols[4], cols[5],
            np.asarray(inputs["atti_b"], np.float32) * WS,
            np.asarray(inputs["fc1_b"], np.float32) * 2.0]
    shared["bias_pack"] = np.ascontiguousarray(np.stack(cols, axis=1))
    shared["atti_w8"] = (np.asarray(inputs["atti_w"], np.float32) * WS
                         ).astype(F8N)
    shared["attsw8"] = (np.asarray(inputs["atts_w"], np.float32) * WS
                        ).astype(F8N)
    shared["fc1_w"] = np.asarray(inputs["fc1_w"], np.float32).astype(BFN)
    shared["fc2_w"] = np.asarray(inputs["fc2_w"], np.float32).astype(BFN)
    shared["fc2_b"] = np.asarray(inputs["fc2_b"], np.float32)
    shared["eye_b"] = np.eye(128, dtype=BFN)

    in_maps = []
    for c in range(N_CORES):
        m = dict(shared)
        m["tick"] = np.zeros((G, 3), np.float32)
        m["xT"] = np.ascontiguousarray(
            x[c * NPC:(c + 1) * NPC].T).astype(F8N)
        m["acnt"] = np.ascontiguousarray(aval[c * G:(c + 1) * G])
        in_maps.append(m)
    return in_maps


def kernel(**inputs):
    nc = _get_program()
    in_maps = make_in_maps(inputs)
    res = run_bass_kernel_spmd(nc, in_maps, core_ids=list(range(N_CORES)))
    out = np.concatenate([res.results[c]["out"] for c in range(N_CORES)], axis=0)
    return out.astype(np.float32)


# revision 66
# speedup vs baseline: 3.8988x; 1.0096x over previous
"""Trainium2 Bass kernel for a 2-layer TransformerConv GNN + attention pooling.

Strategy: 64 equal graphs of 128 nodes; edges are within-graph. Shard 8
graphs per NeuronCore (batch sharding). Per graph, the scatter-softmax
attention over edges is computed DENSELY as masked attention with an edge
multiplicity matrix A[dst, src] (ln counts), so everything is PE matmuls.

All heavy matmuls run in fp8e4m3 with the DoubleRow perf mode (0.5 PE
cycles per output row, two 128-deep k-tiles contracted per instruction).
Weights are pre-scaled by 64 on the host so their fp8 grid is centered
(except ws, kept unit so skip + attention share one PSUM scale); the
extra 64x rides through q/k and is folded into the softmax exp scale
(A is host-prescaled by 4096/SCALE to match). The attention blocks are
software-pipelined (S(b) | P^T(b-1) | skip(b) | PV(b-1)) so softmax
latency hides under PE work; PSUM evacuations alternate DVE/ACT
(GPSIMD cannot access PSUM) with SBUF-side softmax ops on GPSIMD.
The x_q pooling term is contracted via a 0-stride broadcast AP, and
tanh is computed from exp so one activation-table set serves the whole
kernel (pinned up front; no mid-kernel table reloads).
"""

import sys
import numpy as np

if "/opt/trn_rl_repo" not in sys.path:
    sys.path.insert(0, "/opt/trn_rl_repo")

import ml_dtypes  # noqa: F401
import concourse.bacc as bacc
import concourse.bass as bass
import concourse.mybir as mybir
import concourse.tile as tile
from concourse.bass_utils import run_bass_kernel_spmd

F32 = mybir.dt.float32
F32R = mybir.dt.float32r
BF16 = mybir.dt.bfloat16
F8 = mybir.dt.float8e4
AF = mybir.ActivationFunctionType
AX = mybir.AxisListType
ALU = mybir.AluOpType
DR = mybir.MatmulPerfMode.DoubleRow

# problem constants (hardcoded per contract)
B, L, D, H, E = 64, 128, 768, 2, 131072
N = B * L                 # 8192 nodes
C = D // H                # 384 per-head channels
N_CORES = 8
G = B // N_CORES          # 8 graphs per core
NPC = G * L               # 1024 nodes per core
KT = D // 128             # 6 k-tiles of 128
KP = KT // 2              # 3 k-tile pairs (DoubleRow)
CT = C // 128             # 3 c-tiles per head
NCH = NPC // 512          # 2 node chunks of 512
SCALE = 1.0 / float(np.sqrt(C))
WS = 64.0                 # fp8 weight/activation scale carried by q/k
EXP_SCALE = SCALE / (WS * WS)
AMASK = -1e35

_CACHE = {}


def _build_program(repeat=1):
    nc = bacc.Bacc("TRN2", target_bir_lowering=False)

    # ---- DRAM I/O ----
    xT_d = nc.dram_tensor("xT", [D, NPC], F8, kind="ExternalInput")
    a_d = nc.dram_tensor("acnt", [G, L, L], BF16, kind="ExternalInput")
    wd = {}
    for l in ("1", "2"):
        for w in ("wq", "wk", "wv", "ws"):
            wd[w + l] = nc.dram_tensor(w + l, [D, D], F8, kind="ExternalInput")
    # all per-channel bias columns in one DMA (HWDGE costs ~625ns per DMA)
    # order: bq64_1 bk64_1 bsv_1 bq64_2 bk64_2 bsv_2 attib64 fc1b2
    bias_d = nc.dram_tensor("bias_pack", [D, 8], F32, kind="ExternalInput")
    atti_w8 = nc.dram_tensor("atti_w8", [2 * D, D], F8, kind="ExternalInput")
    attsw8_d = nc.dram_tensor("attsw8", [D, 1], F8, kind="ExternalInput")
    fc1_w = nc.dram_tensor("fc1_w", [D, D], BF16, kind="ExternalInput")
    fc2_w = nc.dram_tensor("fc2_w", [D, 3], BF16, kind="ExternalInput")
    fc2_b = nc.dram_tensor("fc2_b", [3], F32, kind="ExternalInput")
    eye_b = nc.dram_tensor("eye_b", [128, 128], BF16, kind="ExternalInput")
    tick_d = nc.dram_tensor("tick", [G, 3], F32, kind="ExternalInput")
    out_d = nc.dram_tensor("out", [G, 3], F32, kind="ExternalOutput")

    with tile.TileContext(nc) as tc, (
        tc.tile_pool(name="wpool", bufs=10)) as wp, (
        tc.tile_pool(name="act", bufs=2)) as actp, (
        tc.tile_pool(name="qk", bufs=2)) as qkp, (
        tc.tile_pool(name="small", bufs=1)) as sp, (
        tc.tile_pool(name="attn", bufs=6)) as ap_, (
        tc.tile_pool(name="psum", bufs=6, space=bass.MemorySpace.PSUM)) as pp, (
        tc.tile_pool(name="psumS", bufs=2, space=bass.MemorySpace.PSUM)) as pps:

        # ---- PE warmup: keep the array busy (and ramping) during the
        # initial DMA loads so real matmuls start at full p-state. Also
        # preload the one activation table set the kernel uses
        # (natural_log_exp_and_others: copy/identity/relu/exp/ln).
        warm = sp.tile([128, 128], BF16, tag="warm")
        nc.vector.memset(warm[:], 1.0)
        # pin the activation table to natural_log_exp_and_others (set 6:
        # copy/identity/relu/exp/ln) so no mid-kernel reload is needed;
        # the insert_act_table_loads pass would otherwise flap 0 <-> 5.
        lset = mybir.InstLoadActFuncSet(
            name=nc.get_next_instruction_name(), act_func_set_id=6,
            ins=[], outs=[])
        nc.scalar.add_instruction(lset)
        dummy = sp.tile([1, 1], F32, tag="dummy")
        nc.scalar.activation(dummy[:], warm[0:1, 0:1], AF.Ln)
        nc.scalar.activation(dummy[:], warm[0:1, 0:1], AF.Exp)
        for i in range(30):
            psw = pps.tile([128, 128], F32, tag="sbank")
            nc.tensor.matmul(psw[:], warm[:], warm[:], start=True, stop=True)

        # on-chip constants (no DMA)
        ones_row_f = sp.tile([1, 128], F32, tag="ones_row_f")
        nc.vector.memset(ones_row_f[:], 1.0)
        ones_col_f = sp.tile([128, 1], F32, tag="ones_col_f")
        nc.gpsimd.memset(ones_col_f[:], 1.0)

        # ---- input / weight / constant loads, ordered by first use
        # (single DMA each; HWDGE serializes at ~625ns per DMA).
        xT = actp.tile([128, KT, NPC], F8, tag="xin", bufs=1)
        wq1 = wp.tile([128, KT, D], F8, tag="w", name="wq1")
        for kp in range(KP):  # paired so the first chains start early
            nc.sync.dma_start(
                xT[:, 2 * kp:2 * kp + 2, :],
                xT_d[256 * kp:256 * (kp + 1), :].rearrange(
                    "(t p) n -> p t n", p=128))
            nc.sync.dma_start(
                wq1[:, 2 * kp:2 * kp + 2, :],
                wd["wq1"][256 * kp:256 * (kp + 1), :].rearrange(
                    "(t p) d -> p t d", p=128))
        bp = sp.tile([128, KT, 8], F32, tag="bias_pack")
        nc.sync.dma_start(bp[:], bias_d[:, :].rearrange("(t p) c -> p t c", p=128))
        BIAS = {"bq641": 0, "bk641": 1, "bsv1": 2, "bq642": 3, "bk642": 4,
                "bsv2": 5, "attib": 6, "fc1b2": 7}

        def bcol(name, dt):
            i = BIAS[name]
            return bp[:, dt, i:i + 1]

        def load_w8(dram, name, rows=None):
            t = wp.tile([128, KT, D], F8, tag="w", name=name)
            src = dram if rows is None else dram[rows[0]:rows[1], :]
            nc.sync.dma_start(t[:], src.rearrange("(t p) d -> p t d", p=128))
            return t

        wk1 = load_w8(wd["wk1"], "wk1")
        wv1 = load_w8(wd["wv1"], "wv1")
        ws1 = load_w8(wd["ws1"], "ws1")
        A_sb = sp.tile([128, G, L], BF16, tag="acnt")
        nc.sync.dma_start(A_sb[:], a_d[:, :, :].rearrange("g p s -> p g s"))
        eyeB = sp.tile([128, 128], BF16, tag="eyeB")
        nc.sync.dma_start(eyeB[:], eye_b[:, :])
        wq2 = load_w8(wd["wq2"], "wq2")
        wk2 = load_w8(wd["wk2"], "wk2")
        wv2 = load_w8(wd["wv2"], "wv2")
        ws2 = load_w8(wd["ws2"], "ws2")
        attiT = load_w8(atti_w8, "attiT", rows=(0, D))
        attiB = load_w8(atti_w8, "attiB", rows=(D, 2 * D))
        attsw = sp.tile([128, KT, 1], F8, tag="attsw")
        nc.sync.dma_start(attsw[:],
                          attsw8_d[:, :].rearrange("(t p) o -> p t o", p=128))
        fc1w = wp.tile([128, KT, D], BF16, tag="wb", name="fc1w", bufs=1)
        nc.sync.dma_start(fc1w[:], fc1_w[:, :].rearrange("(t p) d -> p t d", p=128))
        fc2w_c = sp.tile([128, KT, 3], BF16, tag="fc2w")
        nc.sync.dma_start(fc2w_c[:],
                          fc2_w[:, :].rearrange("(t p) o -> p t o", p=128))
        fc2b_c = sp.tile([3, 1], F32, tag="fc2b")
        nc.sync.dma_start(fc2b_c[:], fc2_b[:].rearrange("(o a) -> o a", a=1))
        tick_t = sp.tile([G, 3], F32, tag="tick")
        nc.sync.dma_start(tick_t[:], tick_d[:, :])
        # derived constants
        eye8 = sp.tile([128, 128], F8, tag="eye8")
        nc.vector.tensor_copy(eye8[:], eyeB[:])
        eyeF3 = sp.tile([3, 3], F32, tag="eyeF3")
        nc.gpsimd.tensor_copy(eyeF3[:], eyeB[0:3, 0:3])

        # round-robin PSUM evacuation across DVE / ACT (GPSIMD cannot
        # access PSUM on TRN2 — BIR verifier rejects it).
        def evac_bias(i, out_ap, ps, bcol):
            if i % 2 == 0:
                nc.vector.tensor_scalar(out_ap, ps, bcol, None, ALU.add)
            else:
                nc.scalar.activation(out_ap, ps, AF.Identity, bias=bcol)

        def evac_scale(i, out_ap, ps, s):
            if i % 2 == 0:
                nc.vector.tensor_scalar_mul(out_ap, ps, s)
            else:
                nc.scalar.activation(out_ap, ps, AF.Identity, scale=s)

        def forward():
            # =========== one TransformerConv layer ===========
            def conv_layer(lidx, actT, wq, wk, wv, ws):
                l = str(lidx)
                qT = qkp.tile([128, KT, NPC], F8, tag="qk", name=f"qT{l}")
                kT = qkp.tile([128, KT, NPC], F8, tag="qk", name=f"kT{l}")
                v_sb = qkp.tile([128, G, D], BF16, tag="v", name=f"v{l}")

                # --- q/k (transposed, fp8, carrying x64) + v (natural,
                # bf16, unit scale); evacs round-robin over 3 engines.
                # weight-arrival order: all Q (wq), then K, then V — PE is
                # in-order, so a V chain emitted early would head-of-line
                # block on the wv DMA during layer 1.
                ev = 0
                for w_sb, o_sb, bkey in ((wq, qT, "bq64"), (wk, kT, "bk64")):
                    for dt in range(KT):
                        for ch in range(NCH):
                            ps = pp.tile([128, 512], F32, tag="bank")
                            for kp in range(KP):
                                nc.tensor.matmul(
                                    ps[:],
                                    w_sb[:, 2 * kp:2 * kp + 2,
                                         dt * 128:(dt + 1) * 128],
                                    actT[:, 2 * kp:2 * kp + 2,
                                         ch * 512:(ch + 1) * 512],
                                    start=(kp == 0), stop=(kp == KP - 1),
                                    perf_mode=DR)
                            evac_bias(ev, o_sb[:, dt, ch * 512:(ch + 1) * 512],
                                      ps[:], bcol(bkey + l, dt))
                            ev += 1
                for gv in range(G):
                    for chv in range(2):
                        ps = pp.tile([128, 384], F32, tag="bank")
                        for kp in range(KP):
                            nc.tensor.matmul(
                                ps[:],
                                actT[:, 2 * kp:2 * kp + 2,
                                     gv * 128:(gv + 1) * 128],
                                wv[:, 2 * kp:2 * kp + 2,
                                   chv * 384:(chv + 1) * 384],
                                start=(kp == 0), stop=(kp == KP - 1),
                                perf_mode=DR)
                        evac_scale(ev, v_sb[:, gv, chv * 384:(chv + 1) * 384],
                                   ps[:], 1.0 / WS)
                        ev += 1

                # --- attention + skip, software-pipelined across the 4
                # (chunk, head) blocks: PE runs S(b), T(b-1), skip(b),
                # PV(b-1) back to back so the softmax (ACT/DVE) latency of
                # block b is hidden under skip/PV work of its neighbours.
                hT = actp.tile([128, KT, NPC], F8, tag="act", name=f"hT{l}")
                blocks = [(ch, h) for ch in range(NCH) for h in range(H)]

                def emit_S(b):
                    ch, h = blocks[b]
                    d0 = h * CT
                    psS = pps.tile([128, 512], F32, tag="sbank")
                    for gl in range(4):
                        g = ch * 4 + gl
                        gs = slice(g * 128, (g + 1) * 128)
                        sl = slice(gl * 128, (gl + 1) * 128)
                        nc.tensor.matmul(psS[:, sl], qT[:, d0:d0 + 2, gs],
                                         kT[:, d0:d0 + 2, gs],
                                         start=True, stop=False, perf_mode=DR)
                        nc.tensor.matmul(psS[:, sl], qT[:, d0 + 2, gs],
                                         kT[:, d0 + 2, gs],
                                         start=False, stop=False)
                        nc.tensor.matmul(psS[:, sl], A_sb[:, g, :], eyeB[:],
                                         start=False, stop=True)
                    return psS

                def emit_softmax(psS):
                    # one 512-wide exp on ACT; per-graph row sums and the
                    # normalize run on GPSIMD from SBUF (legal), DVE only
                    # does the tiny max+reciprocal.
                    Pt = ap_.tile([128, 512], BF16, tag="P")
                    Pn = ap_.tile([128, 512], BF16, tag="Pn")
                    nc.scalar.activation(Pt[:, 0:256], psS[:, 0:256], AF.Exp,
                                         scale=EXP_SCALE)
                    nc.scalar.activation(Pt[:, 256:512], psS[:, 256:512],
                                         AF.Exp, scale=EXP_SCALE)
                    for gl in range(4):
                        sl = slice(gl * 128, (gl + 1) * 128)
                        Z = ap_.tile([128, 1], F32, tag="Z")
                        nc.vector.tensor_reduce(Z[:], Pt[:, sl], AX.X, ALU.add)
                        nc.vector.tensor_scalar_max(Z[:], Z[:], 1e-30)
                        r = ap_.tile([128, 1], F32, tag="r")
                        nc.vector.reciprocal(r[:], Z[:])
                        nc.gpsimd.tensor_scalar(Pn[:, sl], Pt[:, sl], r[:],
                                                None, ALU.mult)
                    return Pn

                def emit_T(b, Pn):
                    psT = pps.tile([128, 512], BF16, tag="sbank")
                    for gl in range(4):
                        sl = slice(gl * 128, (gl + 1) * 128)
                        nc.tensor.transpose(psT[:, sl], Pn[:, sl], eyeB[:])
                    PT = ap_.tile([128, 512], BF16, tag="PT")
                    nc.scalar.copy(PT[:], psT[:])
                    return PT

                def emit_skip(b):
                    ch, h = blocks[b]
                    d0 = h * CT
                    banks = []
                    for ct in range(CT):
                        dt = d0 + ct
                        ps = pp.tile([128, 512], F32, tag="bank")
                        for kp in range(KP):
                            nc.tensor.matmul(
                                ps[:],
                                ws[:, 2 * kp:2 * kp + 2,
                                   dt * 128:(dt + 1) * 128],
                                actT[:, 2 * kp:2 * kp + 2,
                                     ch * 512:(ch + 1) * 512],
                                start=(kp == 0), stop=False, perf_mode=DR)
                        banks.append(ps)
                    return banks

                def emit_PV_evac(b, PT, banks):
                    ch, h = blocks[b]
                    d0 = h * CT
                    for gl in range(4):
                        g = ch * 4 + gl
                        for ct in range(CT):
                            dti = d0 + ct
                            nc.tensor.matmul(
                                banks[ct][:, gl * 128:(gl + 1) * 128],
                                v_sb[:, g, dti * 128:(dti + 1) * 128],
                                PT[:, gl * 128:(gl + 1) * 128],
                                start=False, stop=(gl == 3))
                    for ct in range(CT):
                        dt = d0 + ct
                        oap = hT[:, dt, ch * 512:(ch + 1) * 512]
                        if ct == 1 or (ct == 2 and b % 2 == 0):
                            nc.vector.tensor_scalar(
                                oap, banks[ct][:],
                                bcol("bsv" + l, dt), 0.0, ALU.add, ALU.max)
                        else:
                            nc.scalar.activation(oap, banks[ct][:], AF.Relu,
                                                 bias=bcol("bsv" + l, dt))

                # iteration order keeps the gating op first in each engine
                # queue: PTcopy(b-1) [DVE/Pool] before softmax(b) [DVE], and
                # exp(b) [ACT] before hT evac(b-1) [ACT].
                prev = None
                for b in range(len(blocks)):
                    psS = emit_S(b)
                    if prev is not None:
                        pb, pPn, pbanks = prev
                        PT = emit_T(pb, pPn)
                        Pn = emit_softmax(psS)
                        banks = emit_skip(b)
                        emit_PV_evac(pb, PT, pbanks)
                    else:
                        Pn = emit_softmax(psS)
                        banks = emit_skip(b)
                    prev = (b, Pn, banks)
                pb, pPn, pbanks = prev
                PT = emit_T(pb, pPn)
                emit_PV_evac(pb, PT, pbanks)
                return hT

            h1T = conv_layer(1, xT, wq1, wk1, wv1, ws1)
            h2T = conv_layer(2, h1T, wq2, wk2, wv2, ws2)

            # =========== attention pooling + head ===========
            h2full = h2T[:]
            pstep = h2full.ap[0][0]

            # h2 natural (bf16) via PE transposes, for pooled = h2^T p.
            # fp8 transpose hardware writes one value per 2-byte cell, so
            # the psum output and the evac read use element-step-2 APs.
            h2n = actp.tile([128, G, D], BF16, tag="h2n", bufs=1)
            ev = 0
            for g in range(G):
                for half, nt in ((0, 4), (1, 2)):
                    psX = pps.tile([128, 2 * nt * 128], F8, tag="sbank")
                    pfull = psX[:]
                    pstepX = pfull.ap[0][0]
                    for j in range(nt):
                        dt = half * 4 + j
                        oap = bass.AP(pfull.tensor, pfull.offset + j * 256,
                                      [[pstepX, 128], [2, 128]])
                        nc.tensor.transpose(
                            oap, h2T[:, dt, g * 128:(g + 1) * 128], eye8[:])
                    rap = bass.AP(pfull.tensor, pfull.offset,
                                  [[pstepX, 128], [256, nt], [2, 128]])
                    evac_scale(ev, h2n[:, g, half * 512:half * 512 + nt * 128],
                               rap, 1.0)
                    ev += 1

            # xc = relu([x_q, h2] @ atti + atti_b)  (x64 scale, fp8).
            # The x_q half contracts against a 0-stride broadcast AP that
            # replays each graph's first-node h2 column 128 times, so no
            # cTb precompute and the evacuation is one wide op per bank.
            xcT = qkp.tile([128, KT, NPC], F8, tag="qk", name="xcT")
            psSc = pp.tile([128, G], F32, tag="bank")
            pooled_sb = sp.tile([128, KT, G], BF16, tag="pooledT")
            z1 = sp.tile([128, KT, G], BF16, tag="z1")
            ev = 0
            for ch in range(NCH):
                for dt in range(KT):
                    ps = pp.tile([128, 512], F32, tag="bank")
                    for kp in range(KP):
                        nc.tensor.matmul(
                            ps[:],
                            attiB[:, 2 * kp:2 * kp + 2, dt * 128:(dt + 1) * 128],
                            h2T[:, 2 * kp:2 * kp + 2, ch * 512:(ch + 1) * 512],
                            start=(kp == 0), stop=False, perf_mode=DR)
                    for kp in range(KP):
                        xq_bcast = bass.AP(
                            h2full.tensor,
                            h2full.offset + 2 * kp * NPC + ch * 512,
                            [[pstep, 128], [NPC, 2], [L, 4], [0, 128]])
                        nc.tensor.matmul(
                            ps[:],
                            attiT[:, 2 * kp:2 * kp + 2, dt * 128:(dt + 1) * 128],
                            xq_bcast,
                            start=False, stop=(kp == KP - 1), perf_mode=DR)
                    oap = xcT[:, dt, ch * 512:(ch + 1) * 512]
                    if ev % 2 == 0:
                        nc.vector.tensor_scalar(
                            oap, ps[:], bcol("attib", dt), 0.0,
                            ALU.add, ALU.max)
                    else:
                        nc.scalar.activation(oap, ps[:], AF.Relu,
                                             bias=bcol("attib", dt))
                    ev += 1
                # scores (x4096), per-chunk softmax and pooled columns so
                # the tail pipeline starts before the other chunk finishes
                for gl in range(4):
                    g = ch * 4 + gl
                    for kp in range(KP):
                        nc.tensor.matmul(psSc[:, g:g + 1],
                                         xcT[:, 2 * kp:2 * kp + 2,
                                             g * 128:(g + 1) * 128],
                                         attsw[:, 2 * kp:2 * kp + 2, :],
                                         start=(kp == 0), stop=(kp == KP - 1),
                                         perf_mode=DR)
                cs = slice(ch * 4, (ch + 1) * 4)
                Es = ap_.tile([128, 4], F32, tag="Es")
                nc.scalar.activation(Es[:], psSc[:, cs], AF.Exp,
                                     scale=1.0 / (WS * WS))
                psZ = pps.tile([1, 4], F32, tag="sbank")
                nc.tensor.matmul(psZ[:], ones_col_f[:], Es[:],
                                 start=True, stop=True)
                rz = ap_.tile([1, 4], F32, tag="rz")
                nc.vector.reciprocal(rz[:], psZ[:])
                psZb = pps.tile([128, 4], F32, tag="sbank")
                nc.tensor.matmul(psZb[:], ones_row_f[:], rz[:],
                                 start=True, stop=True)
                pcol4 = sp.tile([128, 4], BF16, tag=f"pcols{ch}")
                nc.vector.tensor_mul(pcol4[:], Es[:], psZb[:])
                # pooled columns for this chunk's graphs
                psP = pps.tile([128, KT * 4], F32, tag="sbank")
                for dt in range(KT):
                    for gl in range(4):
                        g = ch * 4 + gl
                        nc.tensor.matmul(psP[:, dt * 4 + gl:dt * 4 + gl + 1],
                                         h2n[:, g, dt * 128:(dt + 1) * 128],
                                         pcol4[:, gl:gl + 1],
                                         start=True, stop=True)
                pf = pooled_sb[:]
                oap = bass.AP(pf.tensor, pf.offset + ch * 4,
                              [[pf.ap[0][0], 128], [G, KT], [1, 4]])
                nc.scalar.copy(oap, psP[:])

            # fc1 + tanh (as 1 - 2/(e^{2x}+1): keeps the single exp/ln
            # activation table set)
            for dt in range(KT):
                ps = pps.tile([128, G], F32, tag="sbank")
                for kt in range(KT):
                    nc.tensor.matmul(ps[:],
                                     fc1w[:, kt, dt * 128:(dt + 1) * 128],
                                     pooled_sb[:, kt, :],
                                     start=(kt == 0), stop=(kt == KT - 1))
                e1 = ap_.tile([128, G], F32, tag="e1")
                nc.scalar.activation(e1[:], ps[:], AF.Exp, scale=2.0,
                                     bias=bcol("fc1b2", dt))
                ep = ap_.tile([128, G], F32, tag="ep")
                nc.vector.tensor_scalar_add(ep[:], e1[:], 1.0)
                rc = ap_.tile([128, G], F32, tag="rc")
                nc.vector.reciprocal(rc[:], ep[:])
                nc.vector.tensor_scalar(z1[:, dt, :], rc[:], -2.0, 1.0,
                                        ALU.mult, ALU.add)

            # fc2 -> [3, G]
            psO = pps.tile([3, G], F32, tag="sbank")
            for kt in range(KT):
                nc.tensor.matmul(psO[:], fc2w_c[:, kt, :], z1[:, kt, :],
                                 start=(kt == 0), stop=(kt == KT - 1))
            oT = sp.tile([3, G], F32, tag="oT")
            nc.scalar.activation(oT[:], psO[:], AF.Identity, bias=fc2b_c[:])
            psOt = pps.tile([G, 3], F32, tag="sbank")
            nc.tensor.transpose(psOt[:], oT[:], eyeF3[:])
            # logits are O(1): log_softmax without max-subtraction
            eo = ap_.tile([G, 3], F32, tag="eo")
            zo = ap_.tile([G, 1], F32, tag="zo")
            nc.scalar.activation(eo[:], psOt[:], AF.Exp, accum_out=zo[:])
            lz = ap_.tile([G, 1], F32, tag="lz")
            nc.scalar.activation(lz[:], zo[:], AF.Ln)
            ofin = ap_.tile([G, 3], F32, tag="ofin")
            nc.vector.tensor_scalar(ofin[:], psOt[:], lz[:], None, ALU.subtract)
            nc.sync.dma_start(out_d[:, :], ofin[:])

        for _ in range(repeat):
            forward()

    nc.compile()
    return nc


def _get_program(repeat=1):
    key = ("nc", repeat)
    if key not in _CACHE:
        _CACHE[key] = _build_program(repeat)
    return _CACHE[key]


def make_in_maps(inputs):
    F8N = ml_dtypes.float8_e4m3
    BFN = ml_dtypes.bfloat16
    x = np.asarray(inputs["x"], np.float32)
    ei = np.asarray(inputs["edge_index"])
    src, dst = ei[0].astype(np.int64), ei[1].astype(np.int64)
    # A[graph, src_local, dst_local]: ln(edge count) pre-scaled to cancel the
    # x64 q/k scales inside exp; -1e35 masks non-edges.
    flat = dst * L + (src % L)
    acnt = np.bincount(flat, minlength=N * L).reshape(B, L, L).astype(np.float32)
    with np.errstate(divide="ignore"):
        aval = np.where(acnt > 0, np.log(acnt) * (WS * WS / SCALE),
                        np.float32(AMASK))
    aval = np.ascontiguousarray(aval.transpose(0, 2, 1)).astype(BFN)

    shared = {}
    for l in ("1", "2"):
        for w in ("wq", "wk", "wv"):
            shared[w + l] = (np.asarray(inputs[w + l], np.float32) * WS
                             ).astype(F8N)
        shared["ws" + l] = np.asarray(inputs["ws" + l], np.float32).astype(F8N)
    # bias_pack columns: bq64_1 bk64_1 bsv_1 bq64_2 bk64_2 bsv_2 attib64 fc1b2
    cols = []
    for l in ("1", "2"):
        cols.append(np.asarray(inputs["bq" + l], np.float32) * WS)
        cols.append(np.asarray(inputs["bk" + l], np.float32) * WS)
        cols.append(np.asarray(inputs["bs" + l], np.float32)
                    + np.asarray(inputs["bv" + l], np.float32))
    cols = [cols[0], cols[1], cols[2], cols[3], cols[4], cols[5],
            np.asarray(inputs["atti_b"], np.float32) * WS,
            np.asarray(inputs["fc1_b"], np.float32) * 2.0]
    shared["bias_pack"] = np.ascontiguousarray(np.stack(cols, axis=1))
    shared["atti_w8"] = (np.asarray(inputs["atti_w"], np.float32) * WS
                         ).astype(F8N)
    shared["attsw8"] = (np.asarray(inputs["atts_w"], np.float32) * WS
                        ).astype(F8N)
    shared["fc1_w"] = np.asarray(inputs["fc1_w"], np.float32).astype(BFN)
    shared["fc2_w"] = np.asarray(inputs["fc2_w"], np.float32).astype(BFN)
    shared["fc2_b"] = np.asarray(inputs["fc2_b"], np.float32)
    shared["eye_b"] = np.eye(128, dtype=BFN)

    in_maps = []
    for c in range(N_CORES):
        m = dict(shared)
        m["tick"] = np.zeros((G, 3), np.float32)
        m["xT"] = np.ascontiguousarray(
            x[c * NPC:(c + 1) * NPC].T).astype(F8N)
        m["acnt"] = np.ascontiguousarray(aval[c * G:(c + 1) * G])
        in_maps.append(m)
    return in_maps


def kernel(**inputs):
    nc = _get_program()
    in_maps = make_in_maps(inputs)
    res = run_bass_kernel_spmd(nc, in_maps, core_ids=list(range(N_CORES)))
    out = np.concatenate([res.results[c]["out"] for c in range(N_CORES)], axis=0)
    return out.astype(np.float32)
